# revision 9
# baseline (speedup 1.0000x reference)
"""Trainium2 Bass kernel for BinarySplitDecoder (binary-tree leaf probabilities).

Contract: kernel(x) takes the FULL input x [65536, 1023] fp32 and returns the
FULL output [65536, 1024] fp32 (leaf probabilities of a depth-10 binary split
tree, level-major node ordering).

Sharding: pure data parallel — batch dim split evenly across 8 NeuronCores.

Strategy (fp16 + block layout; memory-bound, ~33.5 MB of HBM I/O per core):
  - Host casts x to fp16 and permutes columns (within each tree level, a
    bit-reversal involution); the device returns fp16 leaves in bit-reversed
    ("block") order, which the host un-permutes + casts back to fp32. The
    2e-2 relative-error gate makes fp16 safe (measured ~1.5e-3).
  - Block layout: each tree step writes left children into a packed lower
    half and right children into a packed upper half (instead of interleaving
    with stride 2). Packed 2-byte operands let every tensor_tensor run in the
    DVE 2x_1p perf mode — 2x throughput; the interleaved store of the fp32
    baseline forced 1x mode.
  - right = cur - left replaces cur * (1 - a): no separate (1 - x) pass.
  - Rows processed in chunks of g*128; partition p / free-group i holds batch
    row off + p*g + i, so every chunk DMA is one contiguous 2D block (one
    descriptor per partition — column-sliced 3D patterns cost ~6x more
    sequencer descriptor-gen time and stall the pipeline).
  - xin bufs=3: loads prefetch two chunks ahead. The framework hoists the
    next chunk's level-0 ops above the current chunk's deep levels in the
    in-order DVE queue, so a late load head-of-line-blocks ready work;
    2-deep prefetch left ~4 us bubbles at the ramp-to-steady transition.
  - The output store is split in halves: the left half (final after the
    level-9 multiply) drains while the subtract computes the right half.
  - Loads issue from the ACT sequencer (HWDGE), stores from SP: each
    sequencer drains in order, so a store's wait must not block loads.
  - Small chunks at both ends shorten the pipeline ramp and the final store
    drain. DVE (2x) and DMA both run ~95% of the steady window; remaining
    cost is fixed framework preamble/teardown (~18 us).
"""

import numpy as np

import concourse.bacc as bacc
import concourse.bass as bass
import concourse.mybir as mybir
from concourse.tile import TileContext
from concourse.bass_utils import run_bass_kernel_spmd

TREE_DEPTH = 10
N_NODES = (1 << TREE_DEPTH) - 1  # 1023
N_LEAVES = 1 << TREE_DEPTH  # 1024
N_CORES = 8
P = 128  # SBUF partitions
H = N_LEAVES // 2  # 512


def _bitrev(n: int, bits: int) -> int:
    r = 0
    for _ in range(bits):
        r = (r << 1) | (n & 1)
        n >>= 1
    return r


def _col_perm() -> np.ndarray:
    """xp[:, base+p] = x[:, base+rev_s(p)]: per-level bit-reversal so the
    block-layout walk consumes alphas from contiguous slices."""
    perm = np.arange(N_NODES)
    for s in range(TREE_DEPTH):
        base = (1 << s) - 1
        for p in range(1 << s):
            perm[base + p] = base + _bitrev(p, s)
    return perm


COL_PERM = _col_perm()
# block position j holds standard leaf rev(j); rev is an involution
OUT_PERM = np.array([_bitrev(m, TREE_DEPTH) for m in range(N_LEAVES)])


def build_nc(rows_per_core: int, G: int = 14) -> bass.Bass:
    """Per-core Bass program: DRAM "x" [rows_per_core, 1023] fp16 (columns
    pre-permuted) -> DRAM "y" [rows_per_core, 1024] fp16 (block leaf order).
    """
    units = rows_per_core // P
    # small chunks at both ends: short pipeline ramp AND short store drain
    chunks = [2, 4, 8] + [G] * ((units - 22) // G) + [6, 2]
    assert sum(chunks) == units, (rows_per_core, chunks)
    f16 = mybir.dt.float16

    nc = bacc.Bacc("TRN2", target_bir_lowering=False, debug=False)
    x = nc.declare_dram_parameter("x", [rows_per_core, N_NODES], f16, isOutput=False)
    y = nc.declare_dram_parameter("y", [rows_per_core, N_LEAVES], f16, isOutput=True)

    def x_view(off, g):
        return x[off : off + g * P, :].rearrange("(p g) n -> p (g n)", g=g, p=P)

    def y_view(off, g, c0, c1):
        return y[off : off + g * P, c0:c1].rearrange("(p g) m -> p g m", g=g, p=P)

    with TileContext(nc) as tc:
        with (
            tc.tile_pool(name="xin", bufs=3) as xp,
            tc.tile_pool(name="out", bufs=2) as outp,
            # bufs=2: with one buffer, chunk c+1's level-0 write must wait
            # for the level-9 reads of chunk c (WAR) — a per-chunk stall.
            tc.tile_pool(name="cur", bufs=2) as curp,
        ):
            off = 0
            for g in chunks:
                xt = xp.tile([P, g, N_NODES], f16, tag="x")
                nc.scalar.dma_start(out=xt[:], in_=x_view(off, g))

                out_t = outp.tile([P, g, N_LEAVES], f16, tag="y")
                cur = None
                for d in range(TREE_DEPTH):
                    L = 1 << d
                    if d == TREE_DEPTH - 1:
                        nxt = out_t
                    else:
                        # ping-pong intermediate levels between two shared
                        # slots (sized by the largest level using each tag)
                        nxt = curp.tile([P, g, 2 * L], f16, tag=f"cur{d % 2}")
                    a = xt[:, :, L - 1 : 2 * L - 1]  # [P, g, L] level-d alphas
                    left = nxt[:, :, 0:L]
                    right = nxt[:, :, L : 2 * L]
                    if d == 0:
                        nc.vector.tensor_copy(out=left, in_=a)
                        nc.vector.tensor_scalar(
                            out=right,
                            in0=a,
                            scalar1=-1.0,
                            scalar2=1.0,
                            op0=mybir.AluOpType.mult,
                            op1=mybir.AluOpType.add,
                        )
                    else:
                        nc.vector.tensor_mul(out=left, in0=cur, in1=a)
                        if d == TREE_DEPTH - 1:
                            # the left half of the leaves is final: start
                            # draining it while the right half is computed
                            nc.sync.dma_start(
                                out=y_view(off, g, 0, H), in_=out_t[:, :, 0:H]
                            )
                        nc.vector.tensor_tensor(
                            out=right, in0=cur, in1=left, op=mybir.AluOpType.subtract
                        )
                    cur = nxt

                nc.sync.dma_start(
                    out=y_view(off, g, H, N_LEAVES), in_=out_t[:, :, H:]
                )
                off += g * P

    nc.compile()
    return nc


def _run(x: np.ndarray, **spmd_kwargs):
    """Shard x, run the Bass kernel on all 8 cores, return (y, BassKernelResults)."""
    x = np.asarray(x, dtype=np.float32)
    B = x.shape[0]
    assert B % N_CORES == 0 and x.shape[1] == N_NODES
    rows_per_core = B // N_CORES

    xh = np.ascontiguousarray(x[:, COL_PERM].astype(np.float16))

    nc = build_nc(rows_per_core)
    core_ids = list(range(N_CORES))
    in_maps = [
        {"x": xh[i * rows_per_core : (i + 1) * rows_per_core]} for i in core_ids
    ]
    res = run_bass_kernel_spmd(nc, in_maps, core_ids, **spmd_kwargs)
    out = np.concatenate([r["y"] for r in res.results], axis=0)
    out = out[:, OUT_PERM].astype(np.float32)
    return out, res


def kernel(x: np.ndarray) -> np.ndarray:
    return _run(x)[0]
